# revision 2
# baseline (speedup 1.0000x reference)
"""Windowed 3D attention v2 — dual-layout PV + fp8 DoubleRow S + ACT-minimal.

Design (per core, 8 windows):
  - QKV bf16 as baseline (q,k into psQ, drained to fp8 tiles; v bf16-aug).
  - S^T per head: 4 fp8 DoubleRow matmuls (zero second chunk) into a
    [128, 2048] f32 PSUM tile; q pre-scaled by 8*scale on host, exp scale
    1/8. ONE exp activation per head (ACT is the global bottleneck:
    12 heads x 2048 cols x 0.83ns = 20.5us/window floor).
  - pm = pe * exp(B^T) split between DVE (2-byte 2x mode) and Pool.
  - PV dual layout: out[n, (h,d)+denom] — lhsT = pm n-slice, rhs = V-aug
    (32 V cols + ones col per head), 33-col matmuls, accumulated over
    m-blocks; denominators land per-partition -> cheap normalize with a
    free-dim-broadcast reciprocal multiply. Two n-halves time-share 2
    PSUM banks (PV01 streams per head; PV23 bursts at window end).
  - y (n-major) -> PE transposes + proj, PSUM via the psQ rotation,
    emission deferred into the next window's early slots for PE order.
"""

import os
import numpy as np
import ml_dtypes

DIM = 384
HEADS = 12
B_WIN = 64
N_TOK = 512
NCORES = 8
WPC = B_WIN // NCORES
D_HEAD = DIM // HEADS     # 32
SCALE = D_HEAD ** -0.5
VW = 33                   # V-aug slot: 32 V cols + 1 ones col

USE_FP8_S = os.environ.get("NO_FP8", "") == ""

LAST_RESULT = None
_CACHE = {}


def _pos_mlp_table(pos_proj_w, pos_proj_b, ln1_g, ln1_b, pos1_w, pos1_b,
                   ln2_g, ln2_b, pos2_w, pos2_b, ln3_g, ln3_b, pos3_w, pos3_b):
    H = W = D = 8
    rh = np.arange(1 - H, H)
    biases = np.stack(np.meshgrid(rh, rh, rh, indexing="ij"))
    biases = biases.reshape(3, -1).T.astype(np.float32)

    def ln(x, g, b):
        m = x.mean(-1, keepdims=True)
        v = x.var(-1, keepdims=True)
        return (x - m) / np.sqrt(v + 1e-5) * g + b

    p = biases @ pos_proj_w + pos_proj_b
    p = np.maximum(ln(p, ln1_g, ln1_b), 0.0) @ pos1_w + pos1_b
    p = np.maximum(ln(p, ln2_g, ln2_b), 0.0) @ pos2_w + pos2_b
    p = np.maximum(ln(p, ln3_g, ln3_b), 0.0) @ pos3_w + pos3_b
    return p.astype(np.float32)


def _rpi():
    H = W = D = 8
    coords = np.stack(np.meshgrid(np.arange(H), np.arange(W), np.arange(D),
                                  indexing="ij")).reshape(3, -1)
    rel = (coords[:, :, None] - coords[:, None, :]).transpose(1, 2, 0)
    rel = rel + np.array([H - 1, W - 1, D - 1])
    rel = rel * np.array([(2 * W - 1) * (2 * D - 1), 2 * D - 1, 1])
    return rel.sum(-1)


def _build():
    import concourse.bass as bass
    import concourse.mybir as mybir
    import concourse.tile as tile

    f32 = mybir.dt.float32
    bf16 = mybir.dt.bfloat16
    fp8 = mybir.dt.float8e4
    Exp = mybir.ActivationFunctionType.Exp
    DR = mybir.MatmulPerfMode.DoubleRow

    qk_dt = fp8 if USE_FP8_S else bf16
    QKW = 1024 if USE_FP8_S else N_TOK

    from concourse import bacc
    nc = bacc.Bacc(None)
    xt_ext = nc.declare_dram_parameter("xt", [WPC, 128, 3, N_TOK], bf16, isOutput=False)
    ebt_ext = nc.declare_dram_parameter("ebt", [128, HEADS, 4, N_TOK], bf16, isOutput=False)
    qkvw_ext = nc.declare_dram_parameter("qkvw", [128, 3, 3 * DIM], bf16, isOutput=False)
    projw_ext = nc.declare_dram_parameter("projw", [128, 3, DIM], bf16, isOutput=False)
    ident_ext = nc.declare_dram_parameter("ident", [128, 128], bf16, isOutput=False)
    yt_ext = nc.declare_dram_parameter("yt", [WPC, 3, 128, N_TOK], f32, isOutput=True)

    # pm-mul split: Pool gets the leading cols (starts after exp0, long op),
    # DVE the rest (fast, right after exp1 so pm completes early)
    PM_DVE = 896

    with tile.TileContext(nc) as tc:
        with (
            tc.tile_pool(name="const", bufs=1) as cpool,
            tc.tile_pool(name="xt", bufs=2) as xtp,
            tc.tile_pool(name="pe", bufs=3) as pep,
            tc.tile_pool(name="pm", bufs=19) as pmp,
            tc.tile_pool(name="rcp", bufs=4) as rcpp,
            tc.tile_pool(name="y", bufs=4) as ynp,
            tc.tile_pool(name="yT", bufs=2) as ytp,
            tc.tile_pool(name="ys", bufs=3) as ysp,
            tc.tile_pool(name="psS", bufs=2, space="PSUM") as psS,
            tc.tile_pool(name="psQ", bufs=2, space="PSUM") as psQ,
            tc.tile_pool(name="psO", bufs=2, space="PSUM") as psO,
        ):
            # ---- persistent constants ----
            qkvw = cpool.tile([128, 3, 3 * DIM], bf16, tag="qkvw", name="qkvw")
            projw = cpool.tile([128, 3, DIM], bf16, tag="projw", name="projw")
            ebt = cpool.tile([128, HEADS, 4, N_TOK], bf16, tag="ebt", name="ebt")
            ident = cpool.tile([128, 128], bf16, tag="ident", name="ident")

            # qk tiles: 2 pipeline sets x [q0..q2, k0..k2]. With fp8, q
            # tiles are [128, 1024] whose col 512: half is DoubleRow zeros.
            qk2 = [[cpool.tile([128, QKW if t < 3 else N_TOK], qk_dt,
                               tag=f"qk{s}_{t}", name=f"qk{s}_{t}")
                    for t in range(6)] for s in range(2)]
            if USE_FP8_S:
                for s in range(2):
                    for t in range(3):
                        nc.gpsimd.memset(qk2[s][t][:, N_TOK:], 0.0)

            # V-aug tiles: 3 sets (deferred PV23 of window b still reads
            # set b%3 while b+2's v-units write) x 4 m-blocks; ones persist.
            vaug2 = [[cpool.tile([128, HEADS, VW], bf16, tag=f"va{s}_{m}",
                                 name=f"va{s}_{m}") for m in range(4)]
                     for s in range(3)]
            for s in range(3):
                for m in range(4):
                    nc.gpsimd.memset(vaug2[s][m][:, :, D_HEAD:], 1.0)

            def emit_xt(b):
                xt = xtp.tile([128, 3, N_TOK], bf16, tag="xt", name="xt")
                nc.sync.dma_start(xt[:], xt_ext[b])
                return xt

            def emit_qkv_unit(b, xt, u):
                """u 0..5: q/k feature block (q: 0-2, k: 3-5); u 6..9: V."""
                qk = qk2[b % 2]
                if u < 6:
                    ps = psQ.tile([128, N_TOK], f32, tag="ps", name="psqk")
                    for c in range(3):
                        nc.tensor.matmul(ps[:], qkvw[:, c, 128 * u:128 * (u + 1)],
                                         xt[:, c, :], start=(c == 0), stop=(c == 2))
                    with nc.allow_low_precision(reason="qk fp8 for S matmul"):
                        nc.vector.tensor_copy(qk[u][:, 0:N_TOK], ps[:])
                else:
                    k = u - 6
                    ps = psQ.tile([128, N_TOK], f32, tag="ps", name="psv")
                    for c in range(3):
                        nc.tensor.matmul(ps[:, 0:DIM], xt[:, c, 128 * k:128 * (k + 1)],
                                         qkvw[:, c, 2 * DIM:3 * DIM],
                                         start=(c == 0), stop=(c == 2))
                    v3 = vaug2[b % 3][k]
                    nc.vector.tensor_copy(
                        v3[:, :, 0:D_HEAD],
                        ps[:, 0:DIM].rearrange("p (h d) -> p h d", d=D_HEAD))

            def emit_s_mm(b, h, st, m):
                """One S^T matmul for m-block m into st cols [512m']."""
                qk = qk2[b % 2]
                g, j = h // 4, h % 4
                qt, kt = qk[g], qk[3 + g]
                if USE_FP8_S:
                    lhsT = (kt[32 * j:32 * (j + 1), 128 * m:128 * (m + 1)]
                            .unsqueeze(1).broadcast_to([32, 2, 128]))
                    rhs = (qt[32 * j:32 * (j + 1), :]
                           .rearrange("p (two n) -> p two n", two=2))
                    nc.tensor.matmul(st[:, N_TOK * (m % 2):N_TOK * (m % 2 + 1)],
                                     lhsT, rhs, start=True, stop=True,
                                     perf_mode=DR, tile_position=(32 * j, 0))
                else:
                    nc.tensor.matmul(st[:, N_TOK * (m % 2):N_TOK * (m % 2 + 1)],
                                     kt[32 * j:32 * (j + 1), 128 * m:128 * (m + 1)],
                                     qt[32 * j:32 * (j + 1), 0:N_TOK],
                                     start=True, stop=True,
                                     tile_position=(32 * j, 0))

            ESC = 0.125 if USE_FP8_S else float(SCALE)

            def emit_head_a(b, h):
                """First half: S m-blocks 0,1 -> exp -> pe[0:1024]; Pool mul
                starts here (2127ns) so it finishes right after exp1."""
                stE = psS.tile([128, 2 * N_TOK], f32, tag="st", name="stE")
                emit_s_mm(b, h, stE, 0)
                emit_s_mm(b, h, stE, 1)
                pe = pep.tile([128, 4 * N_TOK], bf16, tag="pe", name="pe")
                nc.scalar.activation(pe[:, 0:2 * N_TOK], stE[:], Exp, scale=ESC)
                pm = pmp.tile([128, 4 * N_TOK], bf16, tag="pm", name="pm")
                ebth = ebt[:, h].rearrange("p m n -> p (m n)")
                nc.gpsimd.tensor_mul(pm[:, 0:PM_DVE], pe[:, 0:PM_DVE],
                                     ebth[:, 0:PM_DVE])
                return pe, pm

            def emit_head_b(b, h, pe, pm):
                """Second half: S m-blocks 2,3 -> exp -> pe[1024:] + DVE mul
                (fast 656ns: pm complete ~0.7us after exp1)."""
                stO = psS.tile([128, 2 * N_TOK], f32, tag="st", name="stO")
                emit_s_mm(b, h, stO, 2)
                emit_s_mm(b, h, stO, 3)
                nc.scalar.activation(pe[:, 2 * N_TOK:], stO[:], Exp, scale=ESC)
                ebth = ebt[:, h].rearrange("p m n -> p (m n)")
                nc.vector.tensor_mul(pm[:, PM_DVE:], pe[:, PM_DVE:],
                                     ebth[:, PM_DVE:])

            def emit_pv(b, pms, h, nbs, po_t):
                """Dual-PV for head h into po tiles for n-blocks nbs."""
                vaug = vaug2[b % 3]
                for nb in nbs:
                    po = po_t[nb]
                    for m in range(4):
                        nc.tensor.matmul(
                            po[:, VW * h:VW * (h + 1)],
                            pms[h][:, N_TOK * m + 128 * nb:N_TOK * m + 128 * (nb + 1)],
                            vaug[m][:, h, :],
                            start=(m == 0), stop=(m == 3))

            def emit_norm(po_t, nb):
                """reciprocal of denominators + broadcast normalize -> y."""
                po3 = po_t[nb].rearrange("p (h c) -> p h c", c=VW)
                rb = rcpp.tile([128, HEADS], f32, tag="rb", name="rb")
                nc.vector.reciprocal(rb[:], po3[:, :, D_HEAD])
                yn = ynp.tile([128, HEADS, D_HEAD], bf16, tag="yn", name="yn")
                nc.vector.tensor_mul(
                    yn[:], po3[:, :, 0:D_HEAD],
                    rb[:].unsqueeze(2).broadcast_to([128, HEADS, D_HEAD]))
                return yn

            def emit_transpose_nb(yns, yT, nb):
                """3 PE transposes of y(nb) -> yT[:, :, nb*128..]."""
                ytps = psQ.tile([128, N_TOK], f32, tag="ps", name="ytps")
                ytb = ytps[:, 0:192].bitcast(mybir.dt.bfloat16)
                yn2 = yns[nb].rearrange("p h d -> p (h d)")
                for cb in range(3):
                    nc.tensor.transpose(ytb[:, 128 * cb:128 * (cb + 1)],
                                        yn2[:, 128 * cb:128 * (cb + 1)], ident[:])
                nc.vector.tensor_copy(
                    yT[:, :, 128 * nb:128 * (nb + 1)],
                    ytb.rearrange("p (c n) -> p c n", n=128))

            def emit_proj(b, yT, cb):
                py = psQ.tile([128, N_TOK], f32, tag="ps", name="py")
                for g in range(3):
                    nc.tensor.matmul(py[:], projw[:, g, 128 * cb:128 * (cb + 1)],
                                     yT[:, g, :], start=(g == 0), stop=(g == 2))
                ysb = ysp.tile([128, N_TOK], f32, tag="ys", name="ysb")
                nc.vector.tensor_copy(ysb[:], py[:])
                nc.sync.dma_start(yt_ext[b, cb], ysb[:])

            # ---- prologue ----
            # dummy exp loads the ACT Exp table during initial DMA waits
            dummy = cpool.tile([128, 16], bf16, tag="dummy", name="dummy")
            nc.gpsimd.memset(dummy[:], 0.0)
            nc.scalar.activation(dummy[:], dummy[:], Exp, scale=1.0)
            # PE p-state warmup (dependency-free) through the DMA wait
            wps = psQ.tile([128, N_TOK], f32, tag="ps", name="warm")
            for _ in range(40):
                nc.tensor.matmul(wps[0:16, 0:16], dummy[:, 0:16],
                                 dummy[:, 0:16], start=True, stop=True)

            xt0 = emit_xt(0)
            # u0/u3 weight slices first so head 0 can start ASAP
            nc.sync.dma_start(qkvw[:, :, 0:128], qkvw_ext[:, :, 0:128])
            nc.sync.dma_start(qkvw[:, :, 384:512], qkvw_ext[:, :, 384:512])
            nc.sync.dma_start(qkvw[:, :, 512:3 * DIM], qkvw_ext[:, :, 512:3 * DIM])
            nc.sync.dma_start(qkvw[:, :, 128:384], qkvw_ext[:, :, 128:384])
            nc.sync.dma_start(ident[:], ident_ext[:])
            emit_qkv_unit(0, xt0, 0)
            emit_qkv_unit(0, xt0, 3)
            for h in range(2):
                nc.sync.dma_start(ebt[:, h], ebt_ext[:, h])

            # deferred-work FIFO: small closures popped between half-head
            # emissions (1 after half-A, 2 after half-B = 36 pops/window)
            work = []
            # window 0: remaining QKV units (v first: PV01 needs vaug)
            for u in (6, 7, 8, 9, 1, 4, 2, 5):
                work.append(lambda u=u: emit_qkv_unit(0, xt0, u))

            LAG = 8            # PV01(h) for h<=3 in-window at slots 8..11
            LAG_LAST = 4       # last window: drain FIFO fast, tighter lag
            QORD = (0, 3, 6, 7, 8, 9, 1, 4, 2, 5)
            prev = None        # (b-1)'s {"yns","yT"} for inline tr/proj
            xt = xt0

            def pop_work(k):
                for _ in range(k):
                    if work:
                        work.pop(0)()

            for b in range(WPC):
                pms = [None] * HEADS
                po_t = {}
                yns = [None] * 4
                next_xt = None

                def alloc_po():
                    t = psO.tile([128, N_TOK], f32, tag="po", name="po")
                    return t[:, 0:HEADS * VW]

                last = b == WPC - 1
                lag = LAG_LAST if last else LAG
                for s in range(HEADS):
                    if b == 0 and 2 <= s < HEADS:
                        nc.sync.dma_start(ebt[:, s], ebt_ext[:, s])
                    if b == 0 and s == 4:
                        nc.sync.dma_start(projw[:], projw_ext[:])
                    half = emit_head_a(b, s)
                    pms[s] = half[1]
                    pop_work(4 if last else 1)
                    emit_head_b(b, s, *half)
                    pop_work(4 if last else 2)
                    if s == 1 and b + 1 < WPC:
                        next_xt = emit_xt(b + 1)
                    if b + 1 < WPC and 2 <= s <= 11:
                        emit_qkv_unit(b + 1, next_xt, QORD[s - 2])
                    if prev is not None and 6 <= s <= 9:
                        emit_transpose_nb(prev["yns"], prev["yT"], s - 6)
                    if prev is not None and s >= 10:
                        emit_proj(b - 1, prev["yT"], s - 10)
                    if s >= lag:
                        if s == lag:
                            # alloc at first use: all prior-generation po
                            # uses (prev window's PV23/norm23) emitted by now
                            po_t[0] = alloc_po()
                            po_t[1] = alloc_po()
                        emit_pv(b, pms, s - lag, (0, 1), po_t)

                if prev is not None:
                    emit_proj(b - 1, prev["yT"], 2)

                yT = ytp.tile([128, 3, N_TOK], bf16, tag="yT", name="yT")

                def q_pv01(b, pms, po_t, h):
                    return lambda: emit_pv(b, pms, h, (0, 1), po_t)

                def q_pv23(b, pms, po_t, h):
                    return lambda: emit_pv(b, pms, h, (2, 3), po_t)

                def q_norm(po_t, yns, nb):
                    def f():
                        yns[nb] = emit_norm(po_t, nb)
                    return f

                def q_alloc23(po_t):
                    def f():
                        po_t[2] = alloc_po()
                        po_t[3] = alloc_po()
                    return f

                tail = []
                for h in range(HEADS - lag, HEADS):
                    tail.append(q_pv01(b, pms, po_t, h))
                tail.append(q_norm(po_t, yns, 0))
                tail.append(q_norm(po_t, yns, 1))
                tail.append(q_alloc23(po_t))
                for h in range(HEADS):
                    tail.append(q_pv23(b, pms, po_t, h))
                tail.append(q_norm(po_t, yns, 2))
                tail.append(q_norm(po_t, yns, 3))

                if b + 1 < WPC:
                    work.extend(tail)
                    prev = {"yns": yns, "yT": yT}
                    xt = next_xt
                else:
                    # final tail: overlap transposes with the PV23 burst
                    for w in tail[:HEADS - lag + 2]:
                        w()          # PV01 leftovers + norm0 + norm1
                    emit_transpose_nb(yns, yT, 0)
                    emit_transpose_nb(yns, yT, 1)
                    for w in tail[HEADS - lag + 2:]:
                        w()          # alloc23 + PV23 + norm2 + norm3
                    emit_transpose_nb(yns, yT, 2)
                    emit_transpose_nb(yns, yT, 3)
                    for cb in range(3):
                        emit_proj(b, yT, cb)
    nc.compile()
    return nc


def kernel(x, H, W, D, mask, qkv_w, qkv_b, proj_w, proj_b,
           pos_proj_w, pos_proj_b, ln1_g, ln1_b, pos1_w, pos1_b,
           ln2_g, ln2_b, pos2_w, pos2_b, ln3_g, ln3_b, pos3_w, pos3_b):
    global LAST_RESULT
    from concourse.bass_utils import run_bass_kernel_spmd

    x = np.asarray(x, np.float32)
    mask = np.asarray(mask, np.float32)
    qkv_w = np.asarray(qkv_w, np.float32)
    qkv_b = np.asarray(qkv_b, np.float32)
    proj_w = np.asarray(proj_w, np.float32)
    proj_b = np.asarray(proj_b, np.float32)

    pos = _pos_mlp_table(
        np.asarray(pos_proj_w, np.float32), np.asarray(pos_proj_b, np.float32),
        np.asarray(ln1_g, np.float32), np.asarray(ln1_b, np.float32),
        np.asarray(pos1_w, np.float32), np.asarray(pos1_b, np.float32),
        np.asarray(ln2_g, np.float32), np.asarray(ln2_b, np.float32),
        np.asarray(pos2_w, np.float32), np.asarray(pos2_b, np.float32),
        np.asarray(ln3_g, np.float32), np.asarray(ln3_b, np.float32),
        np.asarray(pos3_w, np.float32), np.asarray(pos3_b, np.float32))
    rel_bias = pos[_rpi()]                    # (N, N, HEADS)
    bt = rel_bias.transpose(2, 1, 0)          # (HEADS, m, n) = B^T

    if np.any(mask) or np.any(qkv_b):
        return _numpy_reference(x, mask, qkv_w, qkv_b, proj_w, proj_b, rel_bias)

    ebt = np.exp(bt).reshape(HEADS, 4, 128, N_TOK).transpose(2, 0, 1, 3)
    ebt = np.ascontiguousarray(ebt).astype(ml_dtypes.bfloat16)

    qkv_w_eff = qkv_w.copy()
    if USE_FP8_S:
        qkv_w_eff[:, 0:DIM] *= 8.0 * SCALE
    qkvw_bf = np.ascontiguousarray(
        qkv_w_eff.reshape(3, 128, 3 * DIM).transpose(1, 0, 2)).astype(ml_dtypes.bfloat16)
    projw_bf = np.ascontiguousarray(
        proj_w.reshape(3, 128, DIM).transpose(1, 0, 2)).astype(ml_dtypes.bfloat16)
    ident = np.eye(128, dtype=np.float32).astype(ml_dtypes.bfloat16)

    if "nc" not in _CACHE:
        _CACHE["nc"] = _build()
    nc = _CACHE["nc"]

    in_maps = []
    for c in range(NCORES):
        xs = x[c * WPC:(c + 1) * WPC]
        xs_t = xs.transpose(0, 2, 1).reshape(WPC, 3, 128, N_TOK).transpose(0, 2, 1, 3)
        in_maps.append({
            "xt": np.ascontiguousarray(xs_t).astype(ml_dtypes.bfloat16),
            "ebt": ebt, "qkvw": qkvw_bf, "projw": projw_bf, "ident": ident,
        })
    res = run_bass_kernel_spmd(nc, in_maps, list(range(NCORES)))
    LAST_RESULT = res
    outs = []
    for c in range(NCORES):
        yt = np.asarray(res.results[c]["yt"], np.float32)
        outs.append(yt.reshape(WPC, DIM, N_TOK).transpose(0, 2, 1))
    out = np.concatenate(outs, axis=0) + proj_b[None, None, :]
    return out


def _numpy_reference(x, mask, qkv_w, qkv_b, proj_w, proj_b, rel_bias):
    B_, N, C = x.shape
    h, d = HEADS, D_HEAD
    qkv = (x @ qkv_w + qkv_b).reshape(B_, N, 3, h, d).transpose(2, 0, 3, 1, 4)
    q, k, v = qkv[0] * (d ** -0.5), qkv[1], qkv[2]
    attn = np.einsum("bhnd,bhmd->bhnm", q, k) + rel_bias.transpose(2, 0, 1)[None]
    nG = mask.shape[0]
    attn = (attn.reshape(B_ // nG, nG, h, N, N) + mask[None, :, None]).reshape(B_, h, N, N)
    attn = attn - attn.max(-1, keepdims=True)
    e = np.exp(attn)
    p = e / e.sum(-1, keepdims=True)
    out = np.einsum("bhnm,bhmd->bhnd", p, v).transpose(0, 2, 1, 3).reshape(B_, N, C)
    return (out @ proj_w + proj_b).astype(np.float32)


# revision 3
# speedup vs baseline: 1.0157x; 1.0157x over previous
"""Windowed 3D attention — Trainium2, 8 NeuronCores, ACT-exp-bound design.

Sharding: data-parallel over the window dim B_=64 (8 windows per core).

The schedule is built around the Activation engine, whose exp of the
12x512x512 scores is the irreducible serial floor (~2048 cols/head at
0.83ns/col plus ~400ns/instr overhead -> 2 exp instrs per head on two
alternating [128,1024] PSUM tiles keeps ACT back-to-back):
  - S^T per head: 4 fp8e4 DoubleRow matmuls (the second k-chunk is zeros,
    halving cost vs bf16); q pre-scaled by 8*scale on host, exp scale 1/8.
  - pm = exp(S)*exp(B^T): Pool takes cols [0:896] (issued after the first
    exp half), DVE the rest (fast, so pm completes ~0.7us after exp1).
  - Dual-layout PV: out[n, 12*(32 V + 1 ones)] via 33-col matmuls with
    lhsT = pm n-slices, accumulated over m-blocks; the ones column gives
    per-partition softmax denominators -> reciprocal + free-dim-broadcast
    multiply normalizes in one DVE op per n-block. The 4 n-blocks
    time-share 2 PSUM banks: nb01 streams per head, nb23/normalize/
    transpose/proj are deferred closures consumed in the next window's
    half-head slots (PE queue order keeps the next window's S matmuls
    ahead of tail work so ACT never starves).
  - y (n-major) -> 12 PE transposes -> y^T -> proj -> f32 store; all DMA
    triggers ride the otherwise-idle SP queue.

Host precomputes only the tiny DynamicPosBias MLP table plus layout prep.
"""

import os
import numpy as np
import ml_dtypes

DIM = 384
HEADS = 12
B_WIN = 64
N_TOK = 512
NCORES = 8
WPC = B_WIN // NCORES
D_HEAD = DIM // HEADS     # 32
SCALE = D_HEAD ** -0.5
VW = 33                   # V-aug slot: 32 V cols + 1 ones col

USE_FP8_S = os.environ.get("NO_FP8", "") == ""

LAST_RESULT = None
_CACHE = {}


def _pos_mlp_table(pos_proj_w, pos_proj_b, ln1_g, ln1_b, pos1_w, pos1_b,
                   ln2_g, ln2_b, pos2_w, pos2_b, ln3_g, ln3_b, pos3_w, pos3_b):
    H = W = D = 8
    rh = np.arange(1 - H, H)
    biases = np.stack(np.meshgrid(rh, rh, rh, indexing="ij"))
    biases = biases.reshape(3, -1).T.astype(np.float32)

    def ln(x, g, b):
        m = x.mean(-1, keepdims=True)
        v = x.var(-1, keepdims=True)
        return (x - m) / np.sqrt(v + 1e-5) * g + b

    p = biases @ pos_proj_w + pos_proj_b
    p = np.maximum(ln(p, ln1_g, ln1_b), 0.0) @ pos1_w + pos1_b
    p = np.maximum(ln(p, ln2_g, ln2_b), 0.0) @ pos2_w + pos2_b
    p = np.maximum(ln(p, ln3_g, ln3_b), 0.0) @ pos3_w + pos3_b
    return p.astype(np.float32)


def _rpi():
    H = W = D = 8
    coords = np.stack(np.meshgrid(np.arange(H), np.arange(W), np.arange(D),
                                  indexing="ij")).reshape(3, -1)
    rel = (coords[:, :, None] - coords[:, None, :]).transpose(1, 2, 0)
    rel = rel + np.array([H - 1, W - 1, D - 1])
    rel = rel * np.array([(2 * W - 1) * (2 * D - 1), 2 * D - 1, 1])
    return rel.sum(-1)


def _build():
    import concourse.bass as bass
    import concourse.mybir as mybir
    import concourse.tile as tile

    f32 = mybir.dt.float32
    bf16 = mybir.dt.bfloat16
    fp8 = mybir.dt.float8e4
    Exp = mybir.ActivationFunctionType.Exp
    DR = mybir.MatmulPerfMode.DoubleRow

    qk_dt = fp8 if USE_FP8_S else bf16
    QKW = 1024 if USE_FP8_S else N_TOK

    from concourse import bacc
    nc = bacc.Bacc(None)
    xt_ext = nc.declare_dram_parameter("xt", [WPC, 128, 3, N_TOK], bf16, isOutput=False)
    ebt_ext = nc.declare_dram_parameter("ebt", [128, HEADS, 4, N_TOK], bf16, isOutput=False)
    qkvw_ext = nc.declare_dram_parameter("qkvw", [128, 3, 3 * DIM], bf16, isOutput=False)
    projw_ext = nc.declare_dram_parameter("projw", [128, 3, DIM], bf16, isOutput=False)
    ident_ext = nc.declare_dram_parameter("ident", [128, 128], bf16, isOutput=False)
    yt_ext = nc.declare_dram_parameter("yt", [WPC, 3, 128, N_TOK], f32, isOutput=True)

    # pm-mul split: Pool gets the leading cols (starts after exp0, long op),
    # DVE the rest (fast, right after exp1 so pm completes early)
    PM_DVE = 896

    with tile.TileContext(nc) as tc:
        with (
            tc.tile_pool(name="const", bufs=1) as cpool,
            tc.tile_pool(name="xt", bufs=2) as xtp,
            tc.tile_pool(name="pe", bufs=3) as pep,
            tc.tile_pool(name="pm", bufs=19) as pmp,
            tc.tile_pool(name="rcp", bufs=4) as rcpp,
            tc.tile_pool(name="y", bufs=4) as ynp,
            tc.tile_pool(name="yT", bufs=2) as ytp,
            tc.tile_pool(name="ys", bufs=3) as ysp,
            tc.tile_pool(name="psS", bufs=2, space="PSUM") as psS,
            tc.tile_pool(name="psQ", bufs=2, space="PSUM") as psQ,
            tc.tile_pool(name="psO", bufs=2, space="PSUM") as psO,
        ):
            # ---- persistent constants ----
            qkvw = cpool.tile([128, 3, 3 * DIM], bf16, tag="qkvw", name="qkvw")
            projw = cpool.tile([128, 3, DIM], bf16, tag="projw", name="projw")
            ebt = cpool.tile([128, HEADS, 4, N_TOK], bf16, tag="ebt", name="ebt")
            ident = cpool.tile([128, 128], bf16, tag="ident", name="ident")

            # dummy exp loads the ACT Exp table during initial DMA waits;
            # memset FIRST (on DVE) so the PE warmup isn't gated by the
            # Pool memset queue
            dummy = cpool.tile([128, 16], bf16, tag="dummy", name="dummy")
            nc.vector.memset(dummy[:], 0.0)
            nc.scalar.activation(dummy[:], dummy[:], Exp, scale=1.0)
            wps = psQ.tile([128, N_TOK], f32, tag="ps", name="warm")
            for _ in range(40):
                nc.tensor.matmul(wps[0:16, 0:16], dummy[:, 0:16],
                                 dummy[:, 0:16], start=True, stop=True)

            # qk tiles: 2 pipeline sets x [q0..q2, k0..k2]. With fp8, q
            # tiles are [128, 1024] whose col 512: half is DoubleRow zeros.
            qk2 = [[cpool.tile([128, QKW if t < 3 else N_TOK], qk_dt,
                               tag=f"qk{s}_{t}", name=f"qk{s}_{t}")
                    for t in range(6)] for s in range(2)]
            if USE_FP8_S:
                for s in range(2):
                    for t in range(3):
                        nc.gpsimd.memset(qk2[s][t][:, N_TOK:], 0.0)

            # V-aug tiles: 3 sets (deferred PV23 of window b still reads
            # set b%3 while b+2's v-units write) x 4 m-blocks; ones persist.
            vaug2 = [[cpool.tile([128, HEADS, VW], bf16, tag=f"va{s}_{m}",
                                 name=f"va{s}_{m}") for m in range(4)]
                     for s in range(3)]
            for s in range(3):
                for m in range(4):
                    nc.gpsimd.memset(vaug2[s][m][:, :, D_HEAD:], 1.0)

            def emit_xt(b):
                xt = xtp.tile([128, 3, N_TOK], bf16, tag="xt", name="xt")
                nc.sync.dma_start(xt[:], xt_ext[b])
                return xt

            def emit_qkv_unit(b, xt, u):
                """u 0..5: q/k feature block (q: 0-2, k: 3-5); u 6..9: V."""
                qk = qk2[b % 2]
                if u < 6:
                    ps = psQ.tile([128, N_TOK], f32, tag="ps", name="psqk")
                    for c in range(3):
                        nc.tensor.matmul(ps[:], qkvw[:, c, 128 * u:128 * (u + 1)],
                                         xt[:, c, :], start=(c == 0), stop=(c == 2))
                    with nc.allow_low_precision(reason="qk fp8 for S matmul"):
                        nc.vector.tensor_copy(qk[u][:, 0:N_TOK], ps[:])
                else:
                    k = u - 6
                    ps = psQ.tile([128, N_TOK], f32, tag="ps", name="psv")
                    for c in range(3):
                        nc.tensor.matmul(ps[:, 0:DIM], xt[:, c, 128 * k:128 * (k + 1)],
                                         qkvw[:, c, 2 * DIM:3 * DIM],
                                         start=(c == 0), stop=(c == 2))
                    v3 = vaug2[b % 3][k]
                    nc.vector.tensor_copy(
                        v3[:, :, 0:D_HEAD],
                        ps[:, 0:DIM].rearrange("p (h d) -> p h d", d=D_HEAD))

            def emit_s_mm(b, h, st, m):
                """One S^T matmul for m-block m into st cols [512m']."""
                qk = qk2[b % 2]
                g, j = h // 4, h % 4
                qt, kt = qk[g], qk[3 + g]
                if USE_FP8_S:
                    lhsT = (kt[32 * j:32 * (j + 1), 128 * m:128 * (m + 1)]
                            .unsqueeze(1).broadcast_to([32, 2, 128]))
                    rhs = (qt[32 * j:32 * (j + 1), :]
                           .rearrange("p (two n) -> p two n", two=2))
                    nc.tensor.matmul(st[:, N_TOK * (m % 2):N_TOK * (m % 2 + 1)],
                                     lhsT, rhs, start=True, stop=True,
                                     perf_mode=DR, tile_position=(32 * j, 0))
                else:
                    nc.tensor.matmul(st[:, N_TOK * (m % 2):N_TOK * (m % 2 + 1)],
                                     kt[32 * j:32 * (j + 1), 128 * m:128 * (m + 1)],
                                     qt[32 * j:32 * (j + 1), 0:N_TOK],
                                     start=True, stop=True,
                                     tile_position=(32 * j, 0))

            ESC = 0.125 if USE_FP8_S else float(SCALE)

            def emit_head_a(b, h):
                """First half: S m-blocks 0,1 -> exp -> pe[0:1024]; Pool mul
                starts here (2127ns) so it finishes right after exp1."""
                stE = psS.tile([128, 2 * N_TOK], f32, tag="st", name="stE")
                emit_s_mm(b, h, stE, 0)
                emit_s_mm(b, h, stE, 1)
                pe = pep.tile([128, 4 * N_TOK], bf16, tag="pe", name="pe")
                nc.scalar.activation(pe[:, 0:2 * N_TOK], stE[:], Exp, scale=ESC)
                pm = pmp.tile([128, 4 * N_TOK], bf16, tag="pm", name="pm")
                ebth = ebt[:, h].rearrange("p m n -> p (m n)")
                nc.gpsimd.tensor_mul(pm[:, 0:PM_DVE], pe[:, 0:PM_DVE],
                                     ebth[:, 0:PM_DVE])
                return pe, pm

            def emit_head_b(b, h, pe, pm):
                """Second half: S m-blocks 2,3 -> exp -> pe[1024:] + DVE mul
                (fast 656ns: pm complete ~0.7us after exp1)."""
                stO = psS.tile([128, 2 * N_TOK], f32, tag="st", name="stO")
                emit_s_mm(b, h, stO, 2)
                emit_s_mm(b, h, stO, 3)
                nc.scalar.activation(pe[:, 2 * N_TOK:], stO[:], Exp, scale=ESC)
                ebth = ebt[:, h].rearrange("p m n -> p (m n)")
                nc.vector.tensor_mul(pm[:, PM_DVE:], pe[:, PM_DVE:],
                                     ebth[:, PM_DVE:])

            def emit_pv(b, pms, h, nbs, po_t):
                """Dual-PV for head h into po tiles for n-blocks nbs."""
                vaug = vaug2[b % 3]
                for nb in nbs:
                    po = po_t[nb]
                    for m in range(4):
                        nc.tensor.matmul(
                            po[:, VW * h:VW * (h + 1)],
                            pms[h][:, N_TOK * m + 128 * nb:N_TOK * m + 128 * (nb + 1)],
                            vaug[m][:, h, :],
                            start=(m == 0), stop=(m == 3))

            def emit_norm(po_t, nb):
                """reciprocal of denominators + broadcast normalize -> y."""
                po3 = po_t[nb].rearrange("p (h c) -> p h c", c=VW)
                rb = rcpp.tile([128, HEADS], f32, tag="rb", name="rb")
                nc.vector.reciprocal(rb[:], po3[:, :, D_HEAD])
                yn = ynp.tile([128, HEADS, D_HEAD], bf16, tag="yn", name="yn")
                nc.vector.tensor_mul(
                    yn[:], po3[:, :, 0:D_HEAD],
                    rb[:].unsqueeze(2).broadcast_to([128, HEADS, D_HEAD]))
                return yn

            def emit_transpose_nb(yns, yT, nb):
                """3 PE transposes of y(nb) -> yT[:, :, nb*128..]."""
                ytps = psQ.tile([128, N_TOK], f32, tag="ps", name="ytps")
                ytb = ytps[:, 0:192].bitcast(mybir.dt.bfloat16)
                yn2 = yns[nb].rearrange("p h d -> p (h d)")
                for cb in range(3):
                    nc.tensor.transpose(ytb[:, 128 * cb:128 * (cb + 1)],
                                        yn2[:, 128 * cb:128 * (cb + 1)], ident[:])
                nc.vector.tensor_copy(
                    yT[:, :, 128 * nb:128 * (nb + 1)],
                    ytb.rearrange("p (c n) -> p c n", n=128))

            def emit_proj(b, yT, cb):
                py = psQ.tile([128, N_TOK], f32, tag="ps", name="py")
                for g in range(3):
                    nc.tensor.matmul(py[:], projw[:, g, 128 * cb:128 * (cb + 1)],
                                     yT[:, g, :], start=(g == 0), stop=(g == 2))
                ysb = ysp.tile([128, N_TOK], f32, tag="ys", name="ysb")
                nc.vector.tensor_copy(ysb[:], py[:])
                nc.sync.dma_start(yt_ext[b, cb], ysb[:])

            # ---- prologue ----
            xt0 = emit_xt(0)
            # u0/u3 weight slices first so head 0 can start ASAP
            nc.sync.dma_start(qkvw[:, :, 0:128], qkvw_ext[:, :, 0:128])
            nc.sync.dma_start(qkvw[:, :, 384:512], qkvw_ext[:, :, 384:512])
            nc.sync.dma_start(qkvw[:, :, 512:3 * DIM], qkvw_ext[:, :, 512:3 * DIM])
            nc.sync.dma_start(qkvw[:, :, 128:384], qkvw_ext[:, :, 128:384])
            emit_qkv_unit(0, xt0, 0)
            emit_qkv_unit(0, xt0, 3)
            for h in range(2):
                nc.sync.dma_start(ebt[:, h], ebt_ext[:, h])
            nc.sync.dma_start(ident[:], ident_ext[:])

            # deferred-work FIFO: small closures popped between half-head
            # emissions (1 after half-A, 2 after half-B = 36 pops/window)
            work = []
            # window 0: remaining QKV units (v first: PV01 needs vaug)
            for u in (6, 7, 8, 9, 1, 4, 2, 5):
                work.append(lambda u=u: emit_qkv_unit(0, xt0, u))

            LAG = 8            # PV01(h) for h<=3 in-window at slots 8..11
            LAG_LAST = 4       # last window: drain FIFO fast, tighter lag
            QORD = (0, 3, 6, 7, 8, 9, 1, 4, 2, 5)
            prev = None        # (b-1)'s {"yns","yT"} for inline tr/proj
            xt = xt0

            def pop_work(k):
                for _ in range(k):
                    if work:
                        work.pop(0)()

            for b in range(WPC):
                pms = [None] * HEADS
                po_t = {}
                yns = [None] * 4
                next_xt = None

                def alloc_po():
                    t = psO.tile([128, N_TOK], f32, tag="po", name="po")
                    return t[:, 0:HEADS * VW]

                last = b == WPC - 1
                lag = LAG_LAST if last else LAG
                for s in range(HEADS):
                    if b == 0 and 2 <= s < HEADS:
                        nc.sync.dma_start(ebt[:, s], ebt_ext[:, s])
                    if b == 0 and s == 4:
                        nc.sync.dma_start(projw[:], projw_ext[:])
                    half = emit_head_a(b, s)
                    pms[s] = half[1]
                    if not (b == 0 and s == 0):
                        pop_work(4 if last else 1)
                    emit_head_b(b, s, *half)
                    pop_work(4 if last else 2)
                    if s == 1 and b + 1 < WPC:
                        next_xt = emit_xt(b + 1)
                    if b + 1 < WPC and 2 <= s <= 11:
                        emit_qkv_unit(b + 1, next_xt, QORD[s - 2])
                    trb = 3 if last else 6
                    if prev is not None and trb <= s <= trb + 3:
                        emit_transpose_nb(prev["yns"], prev["yT"], s - trb)
                    if prev is not None and not last and s >= 10:
                        emit_proj(b - 1, prev["yT"], s - 10)
                    if prev is not None and last and 7 <= s <= 9:
                        emit_proj(b - 1, prev["yT"], s - 7)
                    if last and s == 9:
                        po_t[2] = psQ.tile([128, N_TOK], f32, tag="ps",
                                           name="po2q")[:, 0:HEADS * VW]
                        po_t[3] = psQ.tile([128, N_TOK], f32, tag="ps",
                                           name="po3q")[:, 0:HEADS * VW]
                    if last and s >= 10:
                        for h23 in range(3 * (s - 10), 3 * (s - 9)):
                            emit_pv(b, pms, h23, (2, 3), po_t)
                    if s >= lag:
                        if s == lag:
                            # alloc at first use: all prior-generation po
                            # uses (prev window's PV23/norm23) emitted by now
                            po_t[0] = alloc_po()
                            po_t[1] = alloc_po()
                        emit_pv(b, pms, s - lag, (0, 1), po_t)

                if prev is not None:
                    emit_proj(b - 1, prev["yT"], 2)

                yT = ytp.tile([128, 3, N_TOK], bf16, tag="yT", name="yT")

                def q_pv01(b, pms, po_t, h):
                    return lambda: emit_pv(b, pms, h, (0, 1), po_t)

                def q_pv23(b, pms, po_t, h):
                    return lambda: emit_pv(b, pms, h, (2, 3), po_t)

                def q_norm(po_t, yns, nb):
                    def f():
                        yns[nb] = emit_norm(po_t, nb)
                    return f

                def q_alloc23(po_t):
                    def f():
                        po_t[2] = alloc_po()
                        po_t[3] = alloc_po()
                    return f

                tail = []
                for h in range(HEADS - lag, HEADS):
                    tail.append(q_pv01(b, pms, po_t, h))
                tail.append(q_norm(po_t, yns, 0))
                tail.append(q_norm(po_t, yns, 1))
                tail.append(q_alloc23(po_t))
                for h in range(HEADS):
                    tail.append(q_pv23(b, pms, po_t, h))
                tail.append(q_norm(po_t, yns, 2))
                tail.append(q_norm(po_t, yns, 3))

                if b + 1 < WPC:
                    work.extend(tail)
                    prev = {"yns": yns, "yT": yT}
                    xt = next_xt
                else:
                    # final tail: PV23 h0..h5 already streamed at slots 10-11
                    for h in range(HEADS - lag, HEADS):
                        emit_pv(b, pms, h, (0, 1), po_t)
                    for h in range(6, HEADS):
                        emit_pv(b, pms, h, (2, 3), po_t)
                    yns[0] = emit_norm(po_t, 0)
                    yns[2] = emit_norm(po_t, 2)
                    yns[1] = emit_norm(po_t, 1)
                    yns[3] = emit_norm(po_t, 3)
                    for nb in range(4):
                        emit_transpose_nb(yns, yT, nb)
                    for cb in range(3):
                        emit_proj(b, yT, cb)
    nc.compile()
    return nc


def kernel(x, H, W, D, mask, qkv_w, qkv_b, proj_w, proj_b,
           pos_proj_w, pos_proj_b, ln1_g, ln1_b, pos1_w, pos1_b,
           ln2_g, ln2_b, pos2_w, pos2_b, ln3_g, ln3_b, pos3_w, pos3_b):
    global LAST_RESULT
    from concourse.bass_utils import run_bass_kernel_spmd

    x = np.asarray(x, np.float32)
    mask = np.asarray(mask, np.float32)
    qkv_w = np.asarray(qkv_w, np.float32)
    qkv_b = np.asarray(qkv_b, np.float32)
    proj_w = np.asarray(proj_w, np.float32)
    proj_b = np.asarray(proj_b, np.float32)

    pos = _pos_mlp_table(
        np.asarray(pos_proj_w, np.float32), np.asarray(pos_proj_b, np.float32),
        np.asarray(ln1_g, np.float32), np.asarray(ln1_b, np.float32),
        np.asarray(pos1_w, np.float32), np.asarray(pos1_b, np.float32),
        np.asarray(ln2_g, np.float32), np.asarray(ln2_b, np.float32),
        np.asarray(pos2_w, np.float32), np.asarray(pos2_b, np.float32),
        np.asarray(ln3_g, np.float32), np.asarray(ln3_b, np.float32),
        np.asarray(pos3_w, np.float32), np.asarray(pos3_b, np.float32))
    rel_bias = pos[_rpi()]                    # (N, N, HEADS)
    bt = rel_bias.transpose(2, 1, 0)          # (HEADS, m, n) = B^T

    if np.any(mask) or np.any(qkv_b):
        return _numpy_reference(x, mask, qkv_w, qkv_b, proj_w, proj_b, rel_bias)

    ebt = np.exp(bt).reshape(HEADS, 4, 128, N_TOK).transpose(2, 0, 1, 3)
    ebt = np.ascontiguousarray(ebt).astype(ml_dtypes.bfloat16)

    qkv_w_eff = qkv_w.copy()
    if USE_FP8_S:
        qkv_w_eff[:, 0:DIM] *= 8.0 * SCALE
    qkvw_bf = np.ascontiguousarray(
        qkv_w_eff.reshape(3, 128, 3 * DIM).transpose(1, 0, 2)).astype(ml_dtypes.bfloat16)
    projw_bf = np.ascontiguousarray(
        proj_w.reshape(3, 128, DIM).transpose(1, 0, 2)).astype(ml_dtypes.bfloat16)
    ident = np.eye(128, dtype=np.float32).astype(ml_dtypes.bfloat16)

    if "nc" not in _CACHE:
        _CACHE["nc"] = _build()
    nc = _CACHE["nc"]

    in_maps = []
    for c in range(NCORES):
        xs = x[c * WPC:(c + 1) * WPC]
        xs_t = xs.transpose(0, 2, 1).reshape(WPC, 3, 128, N_TOK).transpose(0, 2, 1, 3)
        in_maps.append({
            "xt": np.ascontiguousarray(xs_t).astype(ml_dtypes.bfloat16),
            "ebt": ebt, "qkvw": qkvw_bf, "projw": projw_bf, "ident": ident,
        })
    res = run_bass_kernel_spmd(nc, in_maps, list(range(NCORES)))
    LAST_RESULT = res
    outs = []
    for c in range(NCORES):
        yt = np.asarray(res.results[c]["yt"], np.float32)
        outs.append(yt.reshape(WPC, DIM, N_TOK).transpose(0, 2, 1))
    out = np.concatenate(outs, axis=0) + proj_b[None, None, :]
    return out


def _numpy_reference(x, mask, qkv_w, qkv_b, proj_w, proj_b, rel_bias):
    B_, N, C = x.shape
    h, d = HEADS, D_HEAD
    qkv = (x @ qkv_w + qkv_b).reshape(B_, N, 3, h, d).transpose(2, 0, 3, 1, 4)
    q, k, v = qkv[0] * (d ** -0.5), qkv[1], qkv[2]
    attn = np.einsum("bhnd,bhmd->bhnm", q, k) + rel_bias.transpose(2, 0, 1)[None]
    nG = mask.shape[0]
    attn = (attn.reshape(B_ // nG, nG, h, N, N) + mask[None, :, None]).reshape(B_, h, N, N)
    attn = attn - attn.max(-1, keepdims=True)
    e = np.exp(attn)
    p = e / e.sum(-1, keepdims=True)
    out = np.einsum("bhnm,bhmd->bhnd", p, v).transpose(0, 2, 1, 3).reshape(B_, N, C)
    return (out @ proj_w + proj_b).astype(np.float32)


# revision 4
# speedup vs baseline: 1.0157x; 1.0000x over previous
"""Windowed 3D attention v2 — dual-layout PV + fp8 DoubleRow S + ACT-minimal.

Design (per core, 8 windows):
  - QKV bf16 as baseline (q,k into psQ, drained to fp8 tiles; v bf16-aug).
  - S^T per head: 4 fp8 DoubleRow matmuls (zero second chunk) into a
    [128, 2048] f32 PSUM tile; q pre-scaled by 8*scale on host, exp scale
    1/8. ONE exp activation per head (ACT is the global bottleneck:
    12 heads x 2048 cols x 0.83ns = 20.5us/window floor).
  - pm = pe * exp(B^T) split between DVE (2-byte 2x mode) and Pool.
  - PV dual layout: out[n, (h,d)+denom] — lhsT = pm n-slice, rhs = V-aug
    (32 V cols + ones col per head), 33-col matmuls, accumulated over
    m-blocks; denominators land per-partition -> cheap normalize with a
    free-dim-broadcast reciprocal multiply. Two n-halves time-share 2
    PSUM banks (PV01 streams per head; PV23 bursts at window end).
  - y (n-major) -> PE transposes + proj, PSUM via the psQ rotation,
    emission deferred into the next window's early slots for PE order.
"""

import os
import numpy as np
import ml_dtypes

DIM = 384
HEADS = 12
B_WIN = 64
N_TOK = 512
NCORES = 8
WPC = B_WIN // NCORES
D_HEAD = DIM // HEADS     # 32
SCALE = D_HEAD ** -0.5
VW = 33                   # V-aug slot: 32 V cols + 1 ones col

USE_FP8_S = os.environ.get("NO_FP8", "") == ""

LAST_RESULT = None
_CACHE = {}


def _pos_mlp_table(pos_proj_w, pos_proj_b, ln1_g, ln1_b, pos1_w, pos1_b,
                   ln2_g, ln2_b, pos2_w, pos2_b, ln3_g, ln3_b, pos3_w, pos3_b):
    H = W = D = 8
    rh = np.arange(1 - H, H)
    biases = np.stack(np.meshgrid(rh, rh, rh, indexing="ij"))
    biases = biases.reshape(3, -1).T.astype(np.float32)

    def ln(x, g, b):
        m = x.mean(-1, keepdims=True)
        v = x.var(-1, keepdims=True)
        return (x - m) / np.sqrt(v + 1e-5) * g + b

    p = biases @ pos_proj_w + pos_proj_b
    p = np.maximum(ln(p, ln1_g, ln1_b), 0.0) @ pos1_w + pos1_b
    p = np.maximum(ln(p, ln2_g, ln2_b), 0.0) @ pos2_w + pos2_b
    p = np.maximum(ln(p, ln3_g, ln3_b), 0.0) @ pos3_w + pos3_b
    return p.astype(np.float32)


def _rpi():
    H = W = D = 8
    coords = np.stack(np.meshgrid(np.arange(H), np.arange(W), np.arange(D),
                                  indexing="ij")).reshape(3, -1)
    rel = (coords[:, :, None] - coords[:, None, :]).transpose(1, 2, 0)
    rel = rel + np.array([H - 1, W - 1, D - 1])
    rel = rel * np.array([(2 * W - 1) * (2 * D - 1), 2 * D - 1, 1])
    return rel.sum(-1)


def _build():
    import concourse.bass as bass
    import concourse.mybir as mybir
    import concourse.tile as tile

    f32 = mybir.dt.float32
    bf16 = mybir.dt.bfloat16
    fp8 = mybir.dt.float8e4
    Exp = mybir.ActivationFunctionType.Exp
    DR = mybir.MatmulPerfMode.DoubleRow

    qk_dt = fp8 if USE_FP8_S else bf16
    QKW = 1024 if USE_FP8_S else N_TOK

    from concourse import bacc
    nc = bacc.Bacc(None)
    xt_ext = nc.declare_dram_parameter("xt", [WPC, 128, 3, N_TOK], bf16, isOutput=False)
    ebt_ext = nc.declare_dram_parameter("ebt", [128, HEADS, 4, N_TOK], bf16, isOutput=False)
    qkvw_ext = nc.declare_dram_parameter("qkvw", [128, 3, 3 * DIM], bf16, isOutput=False)
    projw_ext = nc.declare_dram_parameter("projw", [128, 3, DIM], bf16, isOutput=False)
    ident_ext = nc.declare_dram_parameter("ident", [128, 128], bf16, isOutput=False)
    yt_ext = nc.declare_dram_parameter("yt", [WPC, 3, 128, N_TOK], f32, isOutput=True)

    # pm-mul split: Pool gets the leading cols (starts after exp0, long op),
    # DVE the rest (fast, right after exp1 so pm completes early)
    PM_DVE = 896

    with tile.TileContext(nc) as tc:
        with (
            tc.tile_pool(name="const", bufs=1) as cpool,
            tc.tile_pool(name="xt", bufs=2) as xtp,
            tc.tile_pool(name="pe", bufs=3) as pep,
            tc.tile_pool(name="pm", bufs=19) as pmp,
            tc.tile_pool(name="rcp", bufs=4) as rcpp,
            tc.tile_pool(name="y", bufs=4) as ynp,
            tc.tile_pool(name="yT", bufs=2) as ytp,
            tc.tile_pool(name="ys", bufs=3) as ysp,
            tc.tile_pool(name="psS", bufs=2, space="PSUM") as psS,
            tc.tile_pool(name="psQ", bufs=2, space="PSUM") as psQ,
            tc.tile_pool(name="psO", bufs=2, space="PSUM") as psO,
        ):
            # ---- persistent constants ----
            qkvw = cpool.tile([128, 3, 3 * DIM], bf16, tag="qkvw", name="qkvw")
            projw = cpool.tile([128, 3, DIM], bf16, tag="projw", name="projw")
            ebt = cpool.tile([128, HEADS, 4, N_TOK], bf16, tag="ebt", name="ebt")
            ident = cpool.tile([128, 128], bf16, tag="ident", name="ident")

            # dummy exp loads the ACT Exp table during initial DMA waits;
            # memset FIRST (on DVE) so the PE warmup isn't gated by the
            # Pool memset queue
            dummy = cpool.tile([128, 16], bf16, tag="dummy", name="dummy")
            nc.vector.memset(dummy[:], 0.0)
            nc.scalar.activation(dummy[:], dummy[:], Exp, scale=1.0)
            wps = psQ.tile([128, N_TOK], f32, tag="ps", name="warm")
            for _ in range(40):
                nc.tensor.matmul(wps[0:16, 0:16], dummy[:, 0:16],
                                 dummy[:, 0:16], start=True, stop=True)

            # qk tiles: 2 pipeline sets x [q0..q2, k0..k2]. With fp8, q
            # tiles are [128, 1024] whose col 512: half is DoubleRow zeros.
            qk2 = [[cpool.tile([128, QKW if t < 3 else N_TOK], qk_dt,
                               tag=f"qk{s}_{t}", name=f"qk{s}_{t}")
                    for t in range(6)] for s in range(2)]
            if USE_FP8_S:
                for s in range(2):
                    for t in range(3):
                        nc.gpsimd.memset(qk2[s][t][:, N_TOK:], 0.0)

            # V-aug tiles: 3 sets (deferred PV23 of window b still reads
            # set b%3 while b+2's v-units write) x 4 m-blocks; ones persist.
            vaug2 = [[cpool.tile([128, HEADS, VW], bf16, tag=f"va{s}_{m}",
                                 name=f"va{s}_{m}") for m in range(4)]
                     for s in range(3)]
            for s in range(3):
                for m in range(4):
                    nc.gpsimd.memset(vaug2[s][m][:, :, D_HEAD:], 1.0)

            def emit_xt(b):
                xt = xtp.tile([128, 3, N_TOK], bf16, tag="xt", name="xt")
                nc.sync.dma_start(xt[:], xt_ext[b])
                return xt

            def emit_qkv_unit(b, xt, u):
                """u 0..5: q/k feature block (q: 0-2, k: 3-5); u 6..9: V."""
                qk = qk2[b % 2]
                if u < 6:
                    ps = psQ.tile([128, N_TOK], f32, tag="ps", name="psqk")
                    for c in range(3):
                        nc.tensor.matmul(ps[:], qkvw[:, c, 128 * u:128 * (u + 1)],
                                         xt[:, c, :], start=(c == 0), stop=(c == 2))
                    with nc.allow_low_precision(reason="qk fp8 for S matmul"):
                        nc.vector.tensor_copy(qk[u][:, 0:N_TOK], ps[:])
                else:
                    k = u - 6
                    ps = psQ.tile([128, N_TOK], f32, tag="ps", name="psv")
                    for c in range(3):
                        nc.tensor.matmul(ps[:, 0:DIM], xt[:, c, 128 * k:128 * (k + 1)],
                                         qkvw[:, c, 2 * DIM:3 * DIM],
                                         start=(c == 0), stop=(c == 2))
                    v3 = vaug2[b % 3][k]
                    nc.vector.tensor_copy(
                        v3[:, :, 0:D_HEAD],
                        ps[:, 0:DIM].rearrange("p (h d) -> p h d", d=D_HEAD))

            def emit_s_mm(b, h, st, m):
                """One S^T matmul for m-block m into st cols [512m']."""
                qk = qk2[b % 2]
                g, j = h // 4, h % 4
                qt, kt = qk[g], qk[3 + g]
                if USE_FP8_S:
                    lhsT = (kt[32 * j:32 * (j + 1), 128 * m:128 * (m + 1)]
                            .unsqueeze(1).broadcast_to([32, 2, 128]))
                    rhs = (qt[32 * j:32 * (j + 1), :]
                           .rearrange("p (two n) -> p two n", two=2))
                    nc.tensor.matmul(st[:, N_TOK * (m % 2):N_TOK * (m % 2 + 1)],
                                     lhsT, rhs, start=True, stop=True,
                                     perf_mode=DR, tile_position=(32 * j, 0))
                else:
                    nc.tensor.matmul(st[:, N_TOK * (m % 2):N_TOK * (m % 2 + 1)],
                                     kt[32 * j:32 * (j + 1), 128 * m:128 * (m + 1)],
                                     qt[32 * j:32 * (j + 1), 0:N_TOK],
                                     start=True, stop=True,
                                     tile_position=(32 * j, 0))

            ESC = 0.125 if USE_FP8_S else float(SCALE)

            def emit_head_a(b, h):
                """First half: S m-blocks 0,1 -> exp -> pe[0:1024]; Pool mul
                starts here (2127ns) so it finishes right after exp1."""
                stE = psS.tile([128, 2 * N_TOK], f32, tag="st", name="stE")
                emit_s_mm(b, h, stE, 0)
                emit_s_mm(b, h, stE, 1)
                pe = pep.tile([128, 4 * N_TOK], bf16, tag="pe", name="pe")
                nc.scalar.activation(pe[:, 0:2 * N_TOK], stE[:], Exp, scale=ESC)
                pm = pmp.tile([128, 4 * N_TOK], bf16, tag="pm", name="pm")
                ebth = ebt[:, h].rearrange("p m n -> p (m n)")
                nc.gpsimd.tensor_mul(pm[:, 0:PM_DVE], pe[:, 0:PM_DVE],
                                     ebth[:, 0:PM_DVE])
                return pe, pm

            def emit_head_b(b, h, pe, pm):
                """Second half: S m-blocks 2,3 -> exp -> pe[1024:] + DVE mul
                (fast 656ns: pm complete ~0.7us after exp1)."""
                stO = psS.tile([128, 2 * N_TOK], f32, tag="st", name="stO")
                emit_s_mm(b, h, stO, 2)
                emit_s_mm(b, h, stO, 3)
                nc.scalar.activation(pe[:, 2 * N_TOK:], stO[:], Exp, scale=ESC)
                ebth = ebt[:, h].rearrange("p m n -> p (m n)")
                nc.vector.tensor_mul(pm[:, PM_DVE:], pe[:, PM_DVE:],
                                     ebth[:, PM_DVE:])

            def emit_pv(b, pms, h, nbs, po_t):
                """Dual-PV for head h into po tiles for n-blocks nbs."""
                vaug = vaug2[b % 3]
                for nb in nbs:
                    po = po_t[nb]
                    for m in range(4):
                        nc.tensor.matmul(
                            po[:, VW * h:VW * (h + 1)],
                            pms[h][:, N_TOK * m + 128 * nb:N_TOK * m + 128 * (nb + 1)],
                            vaug[m][:, h, :],
                            start=(m == 0), stop=(m == 3))

            def emit_norm(po_t, nb):
                """reciprocal of denominators + broadcast normalize -> y."""
                po3 = po_t[nb].rearrange("p (h c) -> p h c", c=VW)
                rb = rcpp.tile([128, HEADS], f32, tag="rb", name="rb")
                nc.vector.reciprocal(rb[:], po3[:, :, D_HEAD])
                yn = ynp.tile([128, HEADS, D_HEAD], bf16, tag="yn", name="yn")
                nc.vector.tensor_mul(
                    yn[:], po3[:, :, 0:D_HEAD],
                    rb[:].unsqueeze(2).broadcast_to([128, HEADS, D_HEAD]))
                return yn

            def emit_transpose_nb(yns, yT, nb, act=False):
                """3 PE transposes of y(nb) -> yT[:, :, nb*128..]."""
                ytps = psQ.tile([128, N_TOK], f32, tag="ps", name="ytps")
                ytb = ytps[:, 0:192].bitcast(mybir.dt.bfloat16)
                yn2 = yns[nb].rearrange("p h d -> p (h d)")
                for cb in range(3):
                    nc.tensor.transpose(ytb[:, 128 * cb:128 * (cb + 1)],
                                        yn2[:, 128 * cb:128 * (cb + 1)], ident[:])
                dst = yT[:, :, 128 * nb:128 * (nb + 1)]
                src = ytb.rearrange("p (c n) -> p c n", n=128)
                if act:
                    nc.scalar.copy(dst, src)
                else:
                    nc.vector.tensor_copy(dst, src)

            def emit_proj(b, yT, cb, act=False, early=False):
                if early:
                    py = psO.tile([128, N_TOK], f32, tag="po", name="pyo")
                else:
                    py = psQ.tile([128, N_TOK], f32, tag="ps", name="py")
                for g in range(3):
                    nc.tensor.matmul(py[:], projw[:, g, 128 * cb:128 * (cb + 1)],
                                     yT[:, g, :], start=(g == 0), stop=(g == 2))
                ysb = ysp.tile([128, N_TOK], f32, tag="ys", name="ysb")
                if act:
                    nc.scalar.copy(ysb[:], py[:])
                else:
                    nc.vector.tensor_copy(ysb[:], py[:])
                nc.sync.dma_start(yt_ext[b, cb], ysb[:])

            # ---- prologue ----
            xt0 = emit_xt(0)
            # u0/u3 weight slices first so head 0 can start ASAP
            nc.sync.dma_start(qkvw[:, :, 0:128], qkvw_ext[:, :, 0:128])
            nc.sync.dma_start(qkvw[:, :, 384:512], qkvw_ext[:, :, 384:512])
            nc.sync.dma_start(qkvw[:, :, 512:3 * DIM], qkvw_ext[:, :, 512:3 * DIM])
            nc.sync.dma_start(qkvw[:, :, 128:384], qkvw_ext[:, :, 128:384])
            emit_qkv_unit(0, xt0, 0)
            emit_qkv_unit(0, xt0, 3)
            for h in range(2):
                nc.sync.dma_start(ebt[:, h], ebt_ext[:, h])
            nc.sync.dma_start(ident[:], ident_ext[:])

            # deferred-work FIFO: small closures popped between half-head
            # emissions (1 after half-A, 2 after half-B = 36 pops/window)
            work = []
            # window 0: remaining QKV units (v first: PV01 needs vaug)
            for u in (6, 7, 8, 9, 1, 4, 2, 5):
                work.append(lambda u=u: emit_qkv_unit(0, xt0, u))

            LAG = 8            # PV01(h) for h<=3 in-window at slots 8..11
            LAG_LAST = 4       # last window: drain FIFO fast, tighter lag
            QORD = (0, 3, 6, 7, 8, 9, 1, 4, 2, 5)
            prev = None        # (b-1)'s {"yns","yT"} for inline tr/proj
            xt = xt0

            def pop_work(k):
                for _ in range(k):
                    if work:
                        work.pop(0)()

            for b in range(WPC):
                pms = [None] * HEADS
                po_t = {}
                yns = [None] * 4
                next_xt = None

                def alloc_po():
                    t = psO.tile([128, N_TOK], f32, tag="po", name="po")
                    return t[:, 0:HEADS * VW]

                last = b == WPC - 1
                lag = LAG_LAST if last else LAG
                for s in range(HEADS):
                    if b == 0 and 2 <= s < HEADS:
                        nc.sync.dma_start(ebt[:, s], ebt_ext[:, s])
                    if b == 0 and s == 4:
                        nc.sync.dma_start(projw[:], projw_ext[:])
                    half = emit_head_a(b, s)
                    pms[s] = half[1]
                    if s != 0:
                        pop_work(4 if last else 1)
                    emit_head_b(b, s, *half)
                    pop_work(4 if last else 2)
                    if s == 1 and b + 1 < WPC:
                        next_xt = emit_xt(b + 1)
                    if b + 1 < WPC and 2 <= s <= 11:
                        emit_qkv_unit(b + 1, next_xt, QORD[s - 2])
                    trb = 2 if last else 6
                    if prev is not None and trb <= s <= trb + 3:
                        emit_transpose_nb(prev["yns"], prev["yT"], s - trb)
                    if prev is not None and not last and s >= 10:
                        emit_proj(b - 1, prev["yT"], s - 10)
                    if prev is not None and last and 6 <= s <= 8:
                        emit_proj(b - 1, prev["yT"], s - 6)
                    if last and s == 8:
                        po_t[2] = psQ.tile([128, N_TOK], f32, tag="ps",
                                           name="po2q")[:, 0:HEADS * VW]
                        po_t[3] = psQ.tile([128, N_TOK], f32, tag="ps",
                                           name="po3q")[:, 0:HEADS * VW]
                    if last and s >= 9:
                        for h23 in range(3 * (s - 9), 3 * (s - 8)):
                            emit_pv(b, pms, h23, (2, 3), po_t)
                    if s >= lag:
                        if s == lag:
                            # alloc at first use: all prior-generation po
                            # uses (prev window's PV23/norm23) emitted by now
                            po_t[0] = alloc_po()
                            po_t[1] = alloc_po()
                        emit_pv(b, pms, s - lag, (0, 1), po_t)

                if prev is not None:
                    emit_proj(b - 1, prev["yT"], 2)

                yT = ytp.tile([128, 3, N_TOK], bf16, tag="yT", name="yT")

                def q_pv01(b, pms, po_t, h):
                    return lambda: emit_pv(b, pms, h, (0, 1), po_t)

                def q_pv23(b, pms, po_t, h):
                    return lambda: emit_pv(b, pms, h, (2, 3), po_t)

                def q_norm(po_t, yns, nb):
                    def f():
                        yns[nb] = emit_norm(po_t, nb)
                    return f

                def q_alloc23(po_t):
                    def f():
                        po_t[2] = alloc_po()
                        po_t[3] = alloc_po()
                    return f

                tail = []
                for h in range(HEADS - lag, HEADS):
                    tail.append(q_pv01(b, pms, po_t, h))
                tail.append(q_norm(po_t, yns, 0))
                tail.append(q_norm(po_t, yns, 1))
                tail.append(q_alloc23(po_t))
                for h in range(HEADS):
                    tail.append(q_pv23(b, pms, po_t, h))
                tail.append(q_norm(po_t, yns, 2))
                tail.append(q_norm(po_t, yns, 3))

                if b + 1 < WPC:
                    work.extend(tail)
                    prev = {"yns": yns, "yT": yT}
                    xt = next_xt
                else:
                    # final tail: PV23 fully streamed in-window; interleave
                    # norms (DVE) with transposes (PE); drains and proj
                    # copies split between the now-idle ACT and DVE
                    for h in range(HEADS - lag, HEADS):
                        emit_pv(b, pms, h, (0, 1), po_t)
                    for h in range(9, HEADS):
                        emit_pv(b, pms, h, (2, 3), po_t)
                    for nb in range(4):
                        yns[nb] = emit_norm(po_t, nb)
                        emit_transpose_nb(yns, yT, nb, act=(nb % 2 == 0))
                    for cb in range(3):
                        emit_proj(b, yT, cb, act=(cb != 1), early=True)
    nc.compile()
    return nc


def kernel(x, H, W, D, mask, qkv_w, qkv_b, proj_w, proj_b,
           pos_proj_w, pos_proj_b, ln1_g, ln1_b, pos1_w, pos1_b,
           ln2_g, ln2_b, pos2_w, pos2_b, ln3_g, ln3_b, pos3_w, pos3_b):
    global LAST_RESULT
    from concourse.bass_utils import run_bass_kernel_spmd

    x = np.asarray(x, np.float32)
    mask = np.asarray(mask, np.float32)
    qkv_w = np.asarray(qkv_w, np.float32)
    qkv_b = np.asarray(qkv_b, np.float32)
    proj_w = np.asarray(proj_w, np.float32)
    proj_b = np.asarray(proj_b, np.float32)

    pos = _pos_mlp_table(
        np.asarray(pos_proj_w, np.float32), np.asarray(pos_proj_b, np.float32),
        np.asarray(ln1_g, np.float32), np.asarray(ln1_b, np.float32),
        np.asarray(pos1_w, np.float32), np.asarray(pos1_b, np.float32),
        np.asarray(ln2_g, np.float32), np.asarray(ln2_b, np.float32),
        np.asarray(pos2_w, np.float32), np.asarray(pos2_b, np.float32),
        np.asarray(ln3_g, np.float32), np.asarray(ln3_b, np.float32),
        np.asarray(pos3_w, np.float32), np.asarray(pos3_b, np.float32))
    rel_bias = pos[_rpi()]                    # (N, N, HEADS)
    bt = rel_bias.transpose(2, 1, 0)          # (HEADS, m, n) = B^T

    if np.any(mask) or np.any(qkv_b):
        return _numpy_reference(x, mask, qkv_w, qkv_b, proj_w, proj_b, rel_bias)

    ebt = np.exp(bt).reshape(HEADS, 4, 128, N_TOK).transpose(2, 0, 1, 3)
    ebt = np.ascontiguousarray(ebt).astype(ml_dtypes.bfloat16)

    qkv_w_eff = qkv_w.copy()
    if USE_FP8_S:
        qkv_w_eff[:, 0:DIM] *= 8.0 * SCALE
    qkvw_bf = np.ascontiguousarray(
        qkv_w_eff.reshape(3, 128, 3 * DIM).transpose(1, 0, 2)).astype(ml_dtypes.bfloat16)
    projw_bf = np.ascontiguousarray(
        proj_w.reshape(3, 128, DIM).transpose(1, 0, 2)).astype(ml_dtypes.bfloat16)
    ident = np.eye(128, dtype=np.float32).astype(ml_dtypes.bfloat16)

    if "nc" not in _CACHE:
        _CACHE["nc"] = _build()
    nc = _CACHE["nc"]

    in_maps = []
    for c in range(NCORES):
        xs = x[c * WPC:(c + 1) * WPC]
        xs_t = xs.transpose(0, 2, 1).reshape(WPC, 3, 128, N_TOK).transpose(0, 2, 1, 3)
        in_maps.append({
            "xt": np.ascontiguousarray(xs_t).astype(ml_dtypes.bfloat16),
            "ebt": ebt, "qkvw": qkvw_bf, "projw": projw_bf, "ident": ident,
        })
    res = run_bass_kernel_spmd(nc, in_maps, list(range(NCORES)))
    LAST_RESULT = res
    outs = []
    for c in range(NCORES):
        yt = np.asarray(res.results[c]["yt"], np.float32)
        outs.append(yt.reshape(WPC, DIM, N_TOK).transpose(0, 2, 1))
    out = np.concatenate(outs, axis=0) + proj_b[None, None, :]
    return out


def _numpy_reference(x, mask, qkv_w, qkv_b, proj_w, proj_b, rel_bias):
    B_, N, C = x.shape
    h, d = HEADS, D_HEAD
    qkv = (x @ qkv_w + qkv_b).reshape(B_, N, 3, h, d).transpose(2, 0, 3, 1, 4)
    q, k, v = qkv[0] * (d ** -0.5), qkv[1], qkv[2]
    attn = np.einsum("bhnd,bhmd->bhnm", q, k) + rel_bias.transpose(2, 0, 1)[None]
    nG = mask.shape[0]
    attn = (attn.reshape(B_ // nG, nG, h, N, N) + mask[None, :, None]).reshape(B_, h, N, N)
    attn = attn - attn.max(-1, keepdims=True)
    e = np.exp(attn)
    p = e / e.sum(-1, keepdims=True)
    out = np.einsum("bhnm,bhmd->bhnd", p, v).transpose(0, 2, 1, 3).reshape(B_, N, C)
    return (out @ proj_w + proj_b).astype(np.float32)


# revision 5
# speedup vs baseline: 1.0192x; 1.0035x over previous
"""Windowed 3D attention v2 — dual-layout PV + fp8 DoubleRow S + ACT-minimal.

Design (per core, 8 windows):
  - QKV bf16 as baseline (q,k into psQ, drained to fp8 tiles; v bf16-aug).
  - S^T per head: 4 fp8 DoubleRow matmuls (zero second chunk) into a
    [128, 2048] f32 PSUM tile; q pre-scaled by 8*scale on host, exp scale
    1/8. ONE exp activation per head (ACT is the global bottleneck:
    12 heads x 2048 cols x 0.83ns = 20.5us/window floor).
  - pm = pe * exp(B^T) split between DVE (2-byte 2x mode) and Pool.
  - PV dual layout: out[n, (h,d)+denom] — lhsT = pm n-slice, rhs = V-aug
    (32 V cols + ones col per head), 33-col matmuls, accumulated over
    m-blocks; denominators land per-partition -> cheap normalize with a
    free-dim-broadcast reciprocal multiply. Two n-halves time-share 2
    PSUM banks (PV01 streams per head; PV23 bursts at window end).
  - y (n-major) -> PE transposes + proj, PSUM via the psQ rotation,
    emission deferred into the next window's early slots for PE order.
"""

import os
import numpy as np
import ml_dtypes

DIM = 384
HEADS = 12
B_WIN = 64
N_TOK = 512
NCORES = 8
WPC = B_WIN // NCORES
D_HEAD = DIM // HEADS     # 32
SCALE = D_HEAD ** -0.5
VW = 33                   # V-aug slot: 32 V cols + 1 ones col

USE_FP8_S = os.environ.get("NO_FP8", "") == ""

LAST_RESULT = None
_CACHE = {}


def _pos_mlp_table(pos_proj_w, pos_proj_b, ln1_g, ln1_b, pos1_w, pos1_b,
                   ln2_g, ln2_b, pos2_w, pos2_b, ln3_g, ln3_b, pos3_w, pos3_b):
    H = W = D = 8
    rh = np.arange(1 - H, H)
    biases = np.stack(np.meshgrid(rh, rh, rh, indexing="ij"))
    biases = biases.reshape(3, -1).T.astype(np.float32)

    def ln(x, g, b):
        m = x.mean(-1, keepdims=True)
        v = x.var(-1, keepdims=True)
        return (x - m) / np.sqrt(v + 1e-5) * g + b

    p = biases @ pos_proj_w + pos_proj_b
    p = np.maximum(ln(p, ln1_g, ln1_b), 0.0) @ pos1_w + pos1_b
    p = np.maximum(ln(p, ln2_g, ln2_b), 0.0) @ pos2_w + pos2_b
    p = np.maximum(ln(p, ln3_g, ln3_b), 0.0) @ pos3_w + pos3_b
    return p.astype(np.float32)


def _rpi():
    H = W = D = 8
    coords = np.stack(np.meshgrid(np.arange(H), np.arange(W), np.arange(D),
                                  indexing="ij")).reshape(3, -1)
    rel = (coords[:, :, None] - coords[:, None, :]).transpose(1, 2, 0)
    rel = rel + np.array([H - 1, W - 1, D - 1])
    rel = rel * np.array([(2 * W - 1) * (2 * D - 1), 2 * D - 1, 1])
    return rel.sum(-1)


def _build():
    import concourse.bass as bass
    import concourse.mybir as mybir
    import concourse.tile as tile

    f32 = mybir.dt.float32
    bf16 = mybir.dt.bfloat16
    fp8 = mybir.dt.float8e4
    Exp = mybir.ActivationFunctionType.Exp
    DR = mybir.MatmulPerfMode.DoubleRow

    qk_dt = fp8 if USE_FP8_S else bf16
    QKW = 1024 if USE_FP8_S else N_TOK

    from concourse import bacc
    nc = bacc.Bacc(None)
    xt_ext = nc.declare_dram_parameter("xt", [WPC, 128, 3, N_TOK], bf16, isOutput=False)
    ebt_ext = nc.declare_dram_parameter("ebt", [128, HEADS, 4, N_TOK], bf16, isOutput=False)
    qkvw_ext = nc.declare_dram_parameter("qkvw", [128, 3, 3 * DIM], bf16, isOutput=False)
    projw_ext = nc.declare_dram_parameter("projw", [128, 3, DIM], bf16, isOutput=False)
    ident_ext = nc.declare_dram_parameter("ident", [128, 128], bf16, isOutput=False)
    yt_ext = nc.declare_dram_parameter("yt", [WPC, 3, 128, N_TOK], f32, isOutput=True)

    # pm-mul split: Pool gets the leading cols (starts after exp0, long op),
    # DVE the rest (fast, right after exp1 so pm completes early)
    PM_DVE = 896

    with tile.TileContext(nc) as tc:
        with (
            tc.tile_pool(name="const", bufs=1) as cpool,
            tc.tile_pool(name="xt", bufs=2) as xtp,
            tc.tile_pool(name="pe", bufs=3) as pep,
            tc.tile_pool(name="pm", bufs=19) as pmp,
            tc.tile_pool(name="rcp", bufs=4) as rcpp,
            tc.tile_pool(name="y", bufs=4) as ynp,
            tc.tile_pool(name="yT", bufs=2) as ytp,
            tc.tile_pool(name="ys", bufs=3) as ysp,
            tc.tile_pool(name="psS", bufs=2, space="PSUM") as psS,
            tc.tile_pool(name="psQ", bufs=2, space="PSUM") as psQ,
            tc.tile_pool(name="psO", bufs=2, space="PSUM") as psO,
        ):
            # ---- persistent constants ----
            qkvw = cpool.tile([128, 3, 3 * DIM], bf16, tag="qkvw", name="qkvw")
            projw = cpool.tile([128, 3, DIM], bf16, tag="projw", name="projw")
            ebt = cpool.tile([128, HEADS, 4, N_TOK], bf16, tag="ebt", name="ebt")
            ident = cpool.tile([128, 128], bf16, tag="ident", name="ident")

            # dummy exp loads the ACT Exp table during initial DMA waits;
            # memset FIRST (on DVE) so the PE warmup isn't gated by the
            # Pool memset queue
            dummy = cpool.tile([128, 16], bf16, tag="dummy", name="dummy")
            nc.vector.memset(dummy[:], 0.0)
            nc.scalar.activation(dummy[:], dummy[:], Exp, scale=1.0)
            wps = psQ.tile([128, N_TOK], f32, tag="ps", name="warm")
            for _ in range(40):
                nc.tensor.matmul(wps[0:16, 0:16], dummy[:, 0:16],
                                 dummy[:, 0:16], start=True, stop=True)

            # qk tiles: 2 pipeline sets x [q0..q2, k0..k2]. With fp8, q
            # tiles are [128, 1024] whose col 512: half is DoubleRow zeros.
            qk2 = [[cpool.tile([128, QKW if t < 3 else N_TOK], qk_dt,
                               tag=f"qk{s}_{t}", name=f"qk{s}_{t}")
                    for t in range(6)] for s in range(2)]
            if USE_FP8_S:
                for s in range(2):
                    for t in range(3):
                        nc.gpsimd.memset(qk2[s][t][:, N_TOK:], 0.0)

            # V-aug tiles: 3 sets (deferred PV23 of window b still reads
            # set b%3 while b+2's v-units write) x 4 m-blocks; ones persist.
            vaug2 = [[cpool.tile([128, HEADS, VW], bf16, tag=f"va{s}_{m}",
                                 name=f"va{s}_{m}") for m in range(4)]
                     for s in range(3)]
            for s in range(3):
                for m in range(4):
                    nc.gpsimd.memset(vaug2[s][m][:, :, D_HEAD:], 1.0)

            def emit_xt(b):
                xt = xtp.tile([128, 3, N_TOK], bf16, tag="xt", name="xt")
                nc.sync.dma_start(xt[:], xt_ext[b])
                return xt

            def emit_qkv_unit(b, xt, u, act_drain=False):
                """u 0..5: q/k feature block (q: 0-2, k: 3-5); u 6..9: V."""
                qk = qk2[b % 2]
                if u < 6:
                    ps = psQ.tile([128, N_TOK], f32, tag="ps", name="psqk")
                    for c in range(3):
                        nc.tensor.matmul(ps[:], qkvw[:, c, 128 * u:128 * (u + 1)],
                                         xt[:, c, :], start=(c == 0), stop=(c == 2))
                    with nc.allow_low_precision(reason="qk fp8 for S matmul"):
                        if act_drain:
                            nc.scalar.copy(qk[u][:, 0:N_TOK], ps[:])
                        else:
                            nc.vector.tensor_copy(qk[u][:, 0:N_TOK], ps[:])
                else:
                    k = u - 6
                    ps = psQ.tile([128, N_TOK], f32, tag="ps", name="psv")
                    for c in range(3):
                        nc.tensor.matmul(ps[:, 0:DIM], xt[:, c, 128 * k:128 * (k + 1)],
                                         qkvw[:, c, 2 * DIM:3 * DIM],
                                         start=(c == 0), stop=(c == 2))
                    v3 = vaug2[b % 3][k]
                    nc.vector.tensor_copy(
                        v3[:, :, 0:D_HEAD],
                        ps[:, 0:DIM].rearrange("p (h d) -> p h d", d=D_HEAD))

            def emit_s_mm(b, h, st, m):
                """One S^T matmul for m-block m into st cols [512m']."""
                qk = qk2[b % 2]
                g, j = h // 4, h % 4
                qt, kt = qk[g], qk[3 + g]
                if USE_FP8_S:
                    lhsT = (kt[32 * j:32 * (j + 1), 128 * m:128 * (m + 1)]
                            .unsqueeze(1).broadcast_to([32, 2, 128]))
                    rhs = (qt[32 * j:32 * (j + 1), :]
                           .rearrange("p (two n) -> p two n", two=2))
                    nc.tensor.matmul(st[:, N_TOK * (m % 2):N_TOK * (m % 2 + 1)],
                                     lhsT, rhs, start=True, stop=True,
                                     perf_mode=DR, tile_position=(32 * j, 0))
                else:
                    nc.tensor.matmul(st[:, N_TOK * (m % 2):N_TOK * (m % 2 + 1)],
                                     kt[32 * j:32 * (j + 1), 128 * m:128 * (m + 1)],
                                     qt[32 * j:32 * (j + 1), 0:N_TOK],
                                     start=True, stop=True,
                                     tile_position=(32 * j, 0))

            ESC = 0.125 if USE_FP8_S else float(SCALE)

            def emit_head_a(b, h):
                """First half: S m-blocks 0,1 -> exp -> pe[0:1024]; Pool mul
                starts here (2127ns) so it finishes right after exp1."""
                stE = psS.tile([128, 2 * N_TOK], f32, tag="st", name="stE")
                emit_s_mm(b, h, stE, 0)
                emit_s_mm(b, h, stE, 1)
                pe = pep.tile([128, 4 * N_TOK], bf16, tag="pe", name="pe")
                nc.scalar.activation(pe[:, 0:2 * N_TOK], stE[:], Exp, scale=ESC)
                pm = pmp.tile([128, 4 * N_TOK], bf16, tag="pm", name="pm")
                ebth = ebt[:, h].rearrange("p m n -> p (m n)")
                nc.gpsimd.tensor_mul(pm[:, 0:PM_DVE], pe[:, 0:PM_DVE],
                                     ebth[:, 0:PM_DVE])
                return pe, pm

            def emit_head_b(b, h, pe, pm):
                """Second half: S m-blocks 2,3 -> exp -> pe[1024:] + DVE mul
                (fast 656ns: pm complete ~0.7us after exp1)."""
                stO = psS.tile([128, 2 * N_TOK], f32, tag="st", name="stO")
                emit_s_mm(b, h, stO, 2)
                emit_s_mm(b, h, stO, 3)
                nc.scalar.activation(pe[:, 2 * N_TOK:], stO[:], Exp, scale=ESC)
                ebth = ebt[:, h].rearrange("p m n -> p (m n)")
                nc.vector.tensor_mul(pm[:, PM_DVE:], pe[:, PM_DVE:],
                                     ebth[:, PM_DVE:])

            def emit_pv(b, pms, h, nbs, po_t):
                """Dual-PV for head h into po tiles for n-blocks nbs."""
                vaug = vaug2[b % 3]
                for nb in nbs:
                    po = po_t[nb]
                    for m in range(4):
                        nc.tensor.matmul(
                            po[:, VW * h:VW * (h + 1)],
                            pms[h][:, N_TOK * m + 128 * nb:N_TOK * m + 128 * (nb + 1)],
                            vaug[m][:, h, :],
                            start=(m == 0), stop=(m == 3))

            def emit_norm(po_t, nb):
                """reciprocal of denominators + broadcast normalize -> y."""
                po3 = po_t[nb].rearrange("p (h c) -> p h c", c=VW)
                rb = rcpp.tile([128, HEADS], f32, tag="rb", name="rb")
                nc.vector.reciprocal(rb[:], po3[:, :, D_HEAD])
                yn = ynp.tile([128, HEADS, D_HEAD], bf16, tag="yn", name="yn")
                nc.vector.tensor_mul(
                    yn[:], po3[:, :, 0:D_HEAD],
                    rb[:].unsqueeze(2).broadcast_to([128, HEADS, D_HEAD]))
                return yn

            def emit_transpose_nb(yns, yT, nb, act=False):
                """3 PE transposes of y(nb) -> yT[:, :, nb*128..]."""
                ytps = psQ.tile([128, N_TOK], f32, tag="ps", name="ytps")
                ytb = ytps[:, 0:192].bitcast(mybir.dt.bfloat16)
                yn2 = yns[nb].rearrange("p h d -> p (h d)")
                for cb in range(3):
                    nc.tensor.transpose(ytb[:, 128 * cb:128 * (cb + 1)],
                                        yn2[:, 128 * cb:128 * (cb + 1)], ident[:])
                dst = yT[:, :, 128 * nb:128 * (nb + 1)]
                src = ytb.rearrange("p (c n) -> p c n", n=128)
                if act:
                    nc.scalar.copy(dst, src)
                else:
                    nc.vector.tensor_copy(dst, src)

            def emit_proj(b, yT, cb, act=False, early=False):
                if early:
                    py = psO.tile([128, N_TOK], f32, tag="po", name="pyo")
                else:
                    py = psQ.tile([128, N_TOK], f32, tag="ps", name="py")
                for g in range(3):
                    nc.tensor.matmul(py[:], projw[:, g, 128 * cb:128 * (cb + 1)],
                                     yT[:, g, :], start=(g == 0), stop=(g == 2))
                ysb = ysp.tile([128, N_TOK], f32, tag="ys", name="ysb")
                if act:
                    nc.scalar.copy(ysb[:], py[:])
                else:
                    nc.vector.tensor_copy(ysb[:], py[:])
                nc.sync.dma_start(yt_ext[b, cb], ysb[:])

            # ---- prologue ----
            xt0 = emit_xt(0)
            # u0/u3 weight slices first so head 0 can start ASAP
            nc.sync.dma_start(qkvw[:, :, 0:128], qkvw_ext[:, :, 0:128])
            nc.sync.dma_start(qkvw[:, :, 384:512], qkvw_ext[:, :, 384:512])
            nc.sync.dma_start(qkvw[:, :, 512:3 * DIM], qkvw_ext[:, :, 512:3 * DIM])
            nc.sync.dma_start(qkvw[:, :, 128:384], qkvw_ext[:, :, 128:384])
            emit_qkv_unit(0, xt0, 0)
            emit_qkv_unit(0, xt0, 3, act_drain=True)
            for h in range(2):
                nc.sync.dma_start(ebt[:, h], ebt_ext[:, h])
            nc.sync.dma_start(ident[:], ident_ext[:])

            # deferred-work FIFO: small closures popped between half-head
            # emissions (1 after half-A, 2 after half-B = 36 pops/window)
            work = []
            # window 0: remaining QKV units (v first: PV01 needs vaug)
            for u in (6, 7, 8, 9, 1, 4, 2, 5):
                work.append(lambda u=u: emit_qkv_unit(0, xt0, u))

            LAG = 8            # PV01(h) for h<=3 in-window at slots 8..11
            LAG_LAST = 4       # last window: drain FIFO fast, tighter lag
            QORD = (0, 3, 6, 7, 8, 9, 1, 4, 2, 5)
            prev = None        # (b-1)'s {"yns","yT"} for inline tr/proj
            xt = xt0

            def pop_work(k):
                for _ in range(k):
                    if work:
                        work.pop(0)()

            for b in range(WPC):
                pms = [None] * HEADS
                po_t = {}
                yns = [None] * 4
                next_xt = None

                def alloc_po():
                    t = psO.tile([128, N_TOK], f32, tag="po", name="po")
                    return t[:, 0:HEADS * VW]

                last = b == WPC - 1
                lag = LAG_LAST if last else LAG
                for s in range(HEADS):
                    if b == 0 and 2 <= s < HEADS:
                        nc.sync.dma_start(ebt[:, s], ebt_ext[:, s])
                    if b == 0 and s == 4:
                        nc.sync.dma_start(projw[:], projw_ext[:])
                    half = emit_head_a(b, s)
                    pms[s] = half[1]
                    if s != 0:
                        pop_work(4 if last else 1)
                    emit_head_b(b, s, *half)
                    pop_work(4 if last else 2)
                    if s == 1 and b + 1 < WPC:
                        next_xt = emit_xt(b + 1)
                    if b + 1 < WPC and 2 <= s <= 11:
                        emit_qkv_unit(b + 1, next_xt, QORD[s - 2])
                    trb = 2 if last else 6
                    if prev is not None and trb <= s <= trb + 3:
                        emit_transpose_nb(prev["yns"], prev["yT"], s - trb)
                    if prev is not None and not last and s >= 10:
                        emit_proj(b - 1, prev["yT"], s - 10)
                    if prev is not None and last and 6 <= s <= 8:
                        emit_proj(b - 1, prev["yT"], s - 6)
                    if last and s == 9:
                        po_t[2] = psQ.tile([128, N_TOK], f32, tag="ps",
                                           name="po2q")[:, 0:HEADS * VW]
                        po_t[3] = psQ.tile([128, N_TOK], f32, tag="ps",
                                           name="po3q")[:, 0:HEADS * VW]
                    if last and s >= 10:
                        for h23 in range(3 * (s - 10), 3 * (s - 9)):
                            emit_pv(b, pms, h23, (2, 3), po_t)
                    if s >= lag:
                        if s == lag:
                            # alloc at first use: all prior-generation po
                            # uses (prev window's PV23/norm23) emitted by now
                            po_t[0] = alloc_po()
                            po_t[1] = alloc_po()
                        emit_pv(b, pms, s - lag, (0, 1), po_t)

                if prev is not None:
                    emit_proj(b - 1, prev["yT"], 2)

                yT = ytp.tile([128, 3, N_TOK], bf16, tag="yT", name="yT")

                def q_pv01(b, pms, po_t, h):
                    return lambda: emit_pv(b, pms, h, (0, 1), po_t)

                def q_pv23(b, pms, po_t, h):
                    return lambda: emit_pv(b, pms, h, (2, 3), po_t)

                def q_norm(po_t, yns, nb):
                    def f():
                        yns[nb] = emit_norm(po_t, nb)
                    return f

                def q_alloc23(po_t):
                    def f():
                        po_t[2] = alloc_po()
                        po_t[3] = alloc_po()
                    return f

                tail = []
                for h in range(HEADS - lag, HEADS):
                    tail.append(q_pv01(b, pms, po_t, h))
                tail.append(q_norm(po_t, yns, 0))
                tail.append(q_norm(po_t, yns, 1))
                tail.append(q_alloc23(po_t))
                for h in range(HEADS):
                    tail.append(q_pv23(b, pms, po_t, h))
                tail.append(q_norm(po_t, yns, 2))
                tail.append(q_norm(po_t, yns, 3))

                if b + 1 < WPC:
                    work.extend(tail)
                    prev = {"yns": yns, "yT": yT}
                    xt = next_xt
                else:
                    # final tail: PV23 fully streamed in-window; interleave
                    # norms (DVE) with transposes (PE); drains and proj
                    # copies split between the now-idle ACT and DVE
                    for h in range(HEADS - lag, HEADS):
                        emit_pv(b, pms, h, (0, 1), po_t)
                    for h in range(6, HEADS):
                        emit_pv(b, pms, h, (2, 3), po_t)
                    for nb in range(4):
                        yns[nb] = emit_norm(po_t, nb)
                        emit_transpose_nb(yns, yT, nb, act=(nb % 2 == 0))
                    for cb in range(3):
                        emit_proj(b, yT, cb, act=(cb != 1), early=True)
    nc.compile()
    return nc


def kernel(x, H, W, D, mask, qkv_w, qkv_b, proj_w, proj_b,
           pos_proj_w, pos_proj_b, ln1_g, ln1_b, pos1_w, pos1_b,
           ln2_g, ln2_b, pos2_w, pos2_b, ln3_g, ln3_b, pos3_w, pos3_b):
    global LAST_RESULT
    from concourse.bass_utils import run_bass_kernel_spmd

    x = np.asarray(x, np.float32)
    mask = np.asarray(mask, np.float32)
    qkv_w = np.asarray(qkv_w, np.float32)
    qkv_b = np.asarray(qkv_b, np.float32)
    proj_w = np.asarray(proj_w, np.float32)
    proj_b = np.asarray(proj_b, np.float32)

    pos = _pos_mlp_table(
        np.asarray(pos_proj_w, np.float32), np.asarray(pos_proj_b, np.float32),
        np.asarray(ln1_g, np.float32), np.asarray(ln1_b, np.float32),
        np.asarray(pos1_w, np.float32), np.asarray(pos1_b, np.float32),
        np.asarray(ln2_g, np.float32), np.asarray(ln2_b, np.float32),
        np.asarray(pos2_w, np.float32), np.asarray(pos2_b, np.float32),
        np.asarray(ln3_g, np.float32), np.asarray(ln3_b, np.float32),
        np.asarray(pos3_w, np.float32), np.asarray(pos3_b, np.float32))
    rel_bias = pos[_rpi()]                    # (N, N, HEADS)
    bt = rel_bias.transpose(2, 1, 0)          # (HEADS, m, n) = B^T

    if np.any(mask) or np.any(qkv_b):
        return _numpy_reference(x, mask, qkv_w, qkv_b, proj_w, proj_b, rel_bias)

    ebt = np.exp(bt).reshape(HEADS, 4, 128, N_TOK).transpose(2, 0, 1, 3)
    ebt = np.ascontiguousarray(ebt).astype(ml_dtypes.bfloat16)

    qkv_w_eff = qkv_w.copy()
    if USE_FP8_S:
        qkv_w_eff[:, 0:DIM] *= 8.0 * SCALE
    qkvw_bf = np.ascontiguousarray(
        qkv_w_eff.reshape(3, 128, 3 * DIM).transpose(1, 0, 2)).astype(ml_dtypes.bfloat16)
    projw_bf = np.ascontiguousarray(
        proj_w.reshape(3, 128, DIM).transpose(1, 0, 2)).astype(ml_dtypes.bfloat16)
    ident = np.eye(128, dtype=np.float32).astype(ml_dtypes.bfloat16)

    if "nc" not in _CACHE:
        _CACHE["nc"] = _build()
    nc = _CACHE["nc"]

    in_maps = []
    for c in range(NCORES):
        xs = x[c * WPC:(c + 1) * WPC]
        xs_t = xs.transpose(0, 2, 1).reshape(WPC, 3, 128, N_TOK).transpose(0, 2, 1, 3)
        in_maps.append({
            "xt": np.ascontiguousarray(xs_t).astype(ml_dtypes.bfloat16),
            "ebt": ebt, "qkvw": qkvw_bf, "projw": projw_bf, "ident": ident,
        })
    res = run_bass_kernel_spmd(nc, in_maps, list(range(NCORES)))
    LAST_RESULT = res
    outs = []
    for c in range(NCORES):
        yt = np.asarray(res.results[c]["yt"], np.float32)
        outs.append(yt.reshape(WPC, DIM, N_TOK).transpose(0, 2, 1))
    out = np.concatenate(outs, axis=0) + proj_b[None, None, :]
    return out


def _numpy_reference(x, mask, qkv_w, qkv_b, proj_w, proj_b, rel_bias):
    B_, N, C = x.shape
    h, d = HEADS, D_HEAD
    qkv = (x @ qkv_w + qkv_b).reshape(B_, N, 3, h, d).transpose(2, 0, 3, 1, 4)
    q, k, v = qkv[0] * (d ** -0.5), qkv[1], qkv[2]
    attn = np.einsum("bhnd,bhmd->bhnm", q, k) + rel_bias.transpose(2, 0, 1)[None]
    nG = mask.shape[0]
    attn = (attn.reshape(B_ // nG, nG, h, N, N) + mask[None, :, None]).reshape(B_, h, N, N)
    attn = attn - attn.max(-1, keepdims=True)
    e = np.exp(attn)
    p = e / e.sum(-1, keepdims=True)
    out = np.einsum("bhnm,bhmd->bhnd", p, v).transpose(0, 2, 1, 3).reshape(B_, N, C)
    return (out @ proj_w + proj_b).astype(np.float32)
